# revision 1
# baseline (speedup 1.0000x reference)
"""Trainium2 Bass kernel for nn_MultiHeadAttention (B=2, S=2048, D=1024, H=16).

Sharding: 8 cores = 2 batches x 4 head-groups. Core c handles batch c//4 and
heads [4*(c%4), 4*(c%4)+4). Each core computes its 4 heads' attention plus the
row-slice of the output projection; the host sums the 4 partial outputs per
batch and adds the output bias.

Per-core layout (transpose-free attention):
  - qT/kT computed in [head_dim, seq] layout (contraction over D needs x^T,
    which the host provides), packed 2 heads per 128-partition tile.
  - scoresT[kv, q] = kT.T @ qT per (head, kv-tile, q-chunk); exp on ScalarE
    reading PSUM directly (scale=1/8 folded into the activation).
  - attnT'[d+1, q] = [v | 1]^T.T @ exp_scoresT accumulated over kv in PSUM:
    row 64 collects the softmax denominators for free (ones column in v').
  - recip = exp(-ln(sums)) on ScalarE (same activation table set as exp);
    broadcast across partitions with a f32r ones-column matmul; DVE multiply
    normalizes attnT into bf16 SBUF.
  - out[s, :] = attnT.T @ wo accumulated over the 4 heads (K=64 each).

All matmuls run in bf16 (inputs cast on host) with fp32 PSUM accumulation.
"""

import sys

for _p in ("/opt/trn_rl_repo",):
    if _p not in sys.path:
        sys.path.insert(0, _p)

import numpy as np
import ml_dtypes

BF16 = ml_dtypes.bfloat16

S = 2048          # sequence length
D = 1024          # embed dim
HC = 4            # heads per core
HD = 64           # head dim
DC = HC * HD      # per-core projection width (256)
ST = S // 128     # s-tiles (16)
DT = D // 128     # D-tiles (8)
QC = S // 512     # q-chunks of 512 (4)
NCORES = 8

_PROGRAM = None


def _build_program():
    import concourse.mybir as mybir
    import concourse.tile as tile
    from concourse import bacc

    dt = mybir.dt
    AF = mybir.ActivationFunctionType
    ALU = mybir.AluOpType

    class _Bacc(bacc.Bacc):
        def insert_act_table_loads(self):
            # This kernel only uses Exp and Ln on the scalar engine; steer the
            # table-load pass to the combined natural_log_exp_and_others set
            # (one resident table, zero mid-stream reloads) by blanking the
            # exp-only / ln-only sets. Indices must be preserved, so entries
            # are emptied rather than removed.
            from concourse.hw_specs import get_activation_tables

            has_activation = any(
                isinstance(i, mybir.InstActivation)
                for b in self.main_func.blocks
                for i in b.instructions
            )
            if not has_activation:
                return
            tables = []
            for name, funcs in get_activation_tables(self.m.arch).items():
                if name in ("exp_and_others", "exp_and_friends", "natural_log"):
                    funcs = set()
                tables.append((name, funcs))
            bacc._bass_rust.insert_act_table_loads(self, tables)

    nc = _Bacc()

    xqT = nc.declare_dram_parameter("xqT", [D, S], dt.bfloat16, isOutput=False)
    xkT = nc.declare_dram_parameter("xkT", [D, S], dt.bfloat16, isOutput=False)
    xvT = nc.declare_dram_parameter("xvT", [D, S], dt.bfloat16, isOutput=False)
    wq = nc.declare_dram_parameter("wq", [D, DC], dt.bfloat16, isOutput=False)
    wk = nc.declare_dram_parameter("wk", [D, DC], dt.bfloat16, isOutput=False)
    wv = nc.declare_dram_parameter("wv", [D, DC], dt.bfloat16, isOutput=False)
    wo = nc.declare_dram_parameter("wo", [HD, HC, D], dt.bfloat16, isOutput=False)
    bq = nc.declare_dram_parameter("bq", [128, 2], dt.float32, isOutput=False)
    bk = nc.declare_dram_parameter("bk", [128, 2], dt.float32, isOutput=False)
    bv = nc.declare_dram_parameter("bv", [128, DC], dt.float32, isOutput=False)
    ones = nc.declare_dram_parameter("ones", [128, 128], dt.float32r, isOutput=False)
    out = nc.declare_dram_parameter("out", [S, D], dt.float32, isOutput=True)

    out_t = out.rearrange("(t p) d -> t p d", p=128)

    with tile.TileContext(nc) as tc:
        with (
            tc.tile_pool(name="const", bufs=1) as cp,
            tc.tile_pool(name="xt", bufs=34) as xp,
            tc.tile_pool(name="expp", bufs=24) as ep,
            tc.tile_pool(name="atp", bufs=3) as atp,
            tc.tile_pool(name="rcp", bufs=2) as rcp,
            tc.tile_pool(name="outp", bufs=4) as op_,
            tc.tile_pool(name="pa", bufs=2, space="PSUM") as pa,
            tc.tile_pool(name="pb", bufs=4, space="PSUM") as pb,
        ):
            # ---- constants ----
            wq_sb = cp.tile([128, DT, DC], dt.bfloat16, tag="wq_sb")
            wk_sb = cp.tile([128, DT, DC], dt.bfloat16, tag="wk_sb")
            wv_sb = cp.tile([128, DT, DC], dt.bfloat16, tag="wv_sb")
            wo_sb = cp.tile([HD, HC, D], dt.bfloat16, tag="wo_sb")
            bq_sb = cp.tile([128, 2], dt.float32, tag="bq_sb")
            bk_sb = cp.tile([128, 2], dt.float32, tag="bk_sb")
            bv_sb = cp.tile([128, DC], dt.float32, tag="bv_sb")
            ones_sb = cp.tile([128, 128], dt.float32r, tag="ones_sb")
            # only K/Q weights go ahead of the critical xk/xq input stream;
            # wv/wo/ones are deferred until after the first input halves
            nc.sync.dma_start(wk_sb[:], wk.rearrange("(t p) m -> p t m", p=128))
            nc.sync.dma_start(bk_sb[:], bk[:])

            # q/k/v staged as per-chunk tiles so attention's dependencies are
            # fine-grained (a monolithic tile would stall attention until the
            # last projection write).
            qT_sb = [cp.tile([128, 2, 512], dt.bfloat16, tag=f"qT_sb{i}", name=f"qT_sb{i}") for i in range(QC)]
            kT_sb = [cp.tile([128, 2, 512], dt.bfloat16, tag=f"kT_sb{i}", name=f"kT_sb{i}") for i in range(QC)]
            # v' blocks of 65 per head: v cols 0..63, ones col 64
            v_sb = [cp.tile([128, HC * 65], dt.bfloat16, tag=f"v_sb{i}", name=f"v_sb{i}") for i in range(ST)]
            for st in range(ST):
                nc.vector.memset(v_sb[st][:], 1.0)

            # ---- projection helpers ----
            def load_xhalf(xT, xts, half):
                # half-tiles per D-chunk so projections start after half the
                # input bytes and the exp stream ramps during the DMA window
                xr = xT.rearrange("(t p) s -> p t s", p=128)
                for Dti in range(DT):
                    xtile = xp.tile([128, S // 2], dt.bfloat16, tag="xt",
                                    name=f"xt_{Dti}_{half}")
                    nc.sync.dma_start(
                        xtile[:], xr[:, Dti, half * (S // 2):(half + 1) * (S // 2)])
                    xts[Dti][half] = xtile

            def qk_proj(xts, w_sb, dst, b_sb, qc):
                half, off = qc // 2, (qc % 2) * 512
                for pt in range(2):
                    ps = pb.tile([128, 512], dt.float32, tag="pb", name=f"pp_{qc}_{pt}")
                    for Dti in range(DT):
                        nc.tensor.matmul(
                            ps[:],
                            w_sb[:, Dti, pt * 128:(pt + 1) * 128],
                            xts[Dti][half][:, off:off + 512],
                            start=(Dti == 0),
                            stop=(Dti == DT - 1),
                        )
                    nc.vector.tensor_scalar_add(
                        dst[qc][:, pt, :], ps[:], b_sb[:, pt:pt + 1],
                    )

            def v_proj(xts, st_range):
                for st in st_range:
                    half, off = st // 8, (st % 8) * 128
                    ps = pb.tile([128, DC], dt.float32, tag="pb", name=f"vp_{st}")
                    for Dti in range(DT):
                        nc.tensor.matmul(
                            ps[:],
                            xts[Dti][half][:, off:off + 128],
                            wv_sb[:, Dti, :],
                            start=(Dti == 0),
                            stop=(Dti == DT - 1),
                        )
                    # v_sb block h: cols h*65..h*65+63 = v + bias; col h*65+64 stays 1.0
                    nc.vector.tensor_tensor(
                        v_sb[st].rearrange("p (h c) -> p h c", c=65)[:, :, 0:64],
                        ps.rearrange("p (h d) -> p h d", d=HD),
                        bv_sb.rearrange("p (h d) -> p h d", d=HD),
                        ALU.add,
                    )

            # ---- attention + output projection, software-pipelined over q-chunks:
            # qc's normalize/out-proj tail is traced AFTER qc+1's attention so the
            # next q-chunk's PSUM/ACT stream never waits on the tail chain.
            qc_state = {}

            def scores_exp(qc, kvb, h):
                pt, lo = h // 2, (h % 2) * 64
                scp = pa.tile([128, 1024], dt.float32, tag="pa", name=f"sc_{qc}_{kvb}_{h}")
                for j in range(2):
                    kt = kvb * 2 + j
                    nc.tensor.matmul(
                        scp[:, j * 512:(j + 1) * 512],
                        kT_sb[kt // 4][lo:lo + 64, pt, (kt % 4) * 128:(kt % 4 + 1) * 128],
                        qT_sb[qc][lo:lo + 64, pt, :],
                        start=True,
                        stop=True,
                    )
                ex = ep.tile([128, 1024], dt.bfloat16, tag="ex", name=f"ex_{qc}_{kvb}_{h}")
                nc.scalar.activation(ex[:], scp[:], AF.Exp, scale=0.125)
                return ex

            def attnT_mm(qc, kvb, h, psA, ex):
                for j in range(2):
                    kt = kvb * 2 + j
                    nc.tensor.matmul(
                        psA[h][0:65, :],
                        v_sb[kt][:, h * 65:h * 65 + 65],
                        ex[:, j * 512:(j + 1) * 512],
                        start=(kvb == 0 and j == 0),
                        stop=(kvb == 7 and j == 1),
                    )

            def attention(qc, kvb_range, psA, pre_ex=None):
                for kvb in kvb_range:  # kv blocks of 2 kv-tiles
                    for h in range(HC):
                        key = (kvb, h)
                        if pre_ex and key in pre_ex:
                            ex = pre_ex.pop(key)
                        else:
                            ex = scores_exp(qc, kvb, h)
                        attnT_mm(qc, kvb, h, psA, ex)
                if kvb_range[-1] == 7:
                    # drain PSUM accumulators to SBUF right away (frees the
                    # banks): unnormalized attn rows + sums row
                    rc = rcp.tile([128, HC, 512], dt.float32r, tag="rc")
                    at = atp.tile([64, HC, 512], dt.bfloat16, tag="at")
                    # sums rows first: they gate the reciprocal chain
                    for h in range(HC):
                        nc.vector.tensor_copy(rc[64:65, h, :], psA[h][64:65, :])
                    for h in range(HC):
                        nc.vector.tensor_copy(at[:, h, :], psA[h][0:64, :])
                    qc_state[qc] = (rc, at)

            def tail(qc, last=False):
                rc, at = qc_state.pop(qc)
                # reciprocals: exp(-ln(x)) keeps everything on the exp/ln table set
                nc.scalar.activation(rc[64:65, :, :], rc[64:65, :, :], AF.Ln)
                nc.scalar.activation(rc[64:65, :, :], rc[64:65, :, :], AF.Exp, scale=-1.0)
                for h in range(HC):
                    pbc = pb.tile([128, 512], dt.float32, tag="pb", name=f"bc_{qc}_{h}")
                    nc.tensor.matmul(pbc[:], ones_sb[64:65, :], rc[64:65, h, :],
                                     start=True, stop=True)
                    nc.vector.tensor_tensor(
                        at[:, h, :], at[:, h, :], pbc[0:64, :], ALU.mult,
                    )
                # output projection for this q-chunk's 4 s-tiles
                for sl in range(4):
                    st = qc * 4 + sl
                    o_sb = op_.tile([128, D], dt.float32, tag="osb")
                    for dc2 in range(2):
                        po = pb.tile([128, 512], dt.float32, tag="pb", name=f"po_{st}_{dc2}")
                        for h in range(HC):
                            nc.tensor.matmul(
                                po[:],
                                at[:, h, sl * 128:(sl + 1) * 128],
                                wo_sb[:, h, dc2 * 512:(dc2 + 1) * 512],
                                start=(h == 0),
                                stop=(h == HC - 1),
                            )
                        if last:
                            # end tail: ACT is idle; use it for the copies
                            nc.scalar.copy(o_sb[:, dc2 * 512:(dc2 + 1) * 512], po[:])
                        else:
                            nc.vector.tensor_copy(o_sb[:, dc2 * 512:(dc2 + 1) * 512], po[:])
                        # each half ships as soon as it's staged
                        nc.sync.dma_start(
                            out_t[st][:, dc2 * 512:(dc2 + 1) * 512],
                            o_sb[:, dc2 * 512:(dc2 + 1) * 512])

            # trace order chosen so the exp stream (the ACT bottleneck) starts
            # as early as possible: first halves of xk/xq land first, feeding
            # K(qc0,1)+Q(qc0) and the first half of qc0's scores/exp while the
            # second halves and xv are still streaming; V and the remaining Q
            # chunks fill PE gaps under the ACT-bound attention stream.
            xk_ts = [[None, None] for _ in range(DT)]
            xq_ts = [[None, None] for _ in range(DT)]
            xv_ts = [[None, None] for _ in range(DT)]
            load_xhalf(xkT, xk_ts, 0)
            nc.sync.dma_start(wq_sb[:], wq.rearrange("(t p) m -> p t m", p=128))
            nc.sync.dma_start(bq_sb[:], bq[:])
            load_xhalf(xqT, xq_ts, 0)
            qk_proj(xk_ts, wk_sb, kT_sb, bk_sb, 0)
            qk_proj(xk_ts, wk_sb, kT_sb, bk_sb, 1)
            qk_proj(xq_ts, wq_sb, qT_sb, bq_sb, 0)
            pre_ex = {}
            for kvb in range(4):   # needs only kT_sb[0..1] (xk half 0)
                for h in range(HC):
                    pre_ex[(kvb, h)] = scores_exp(0, kvb, h)
            nc.sync.dma_start(wv_sb[:], wv.rearrange("(t p) m -> p t m", p=128))
            nc.sync.dma_start(bv_sb[:], bv[:])
            nc.sync.dma_start(wo_sb[:], wo[:])
            nc.sync.dma_start(ones_sb[:], ones[:])
            load_xhalf(xvT, xv_ts, 0)
            v_proj(xv_ts, range(0, 8))
            load_xhalf(xkT, xk_ts, 1)
            load_xhalf(xqT, xq_ts, 1)
            qk_proj(xk_ts, wk_sb, kT_sb, bk_sb, 2)
            qk_proj(xk_ts, wk_sb, kT_sb, bk_sb, 3)
            for kvb in range(4, 8):
                for h in range(HC):
                    pre_ex[(kvb, h)] = scores_exp(0, kvb, h)
            for qc in range(1, QC):
                qk_proj(xq_ts, wq_sb, qT_sb, bq_sb, qc)
            load_xhalf(xvT, xv_ts, 1)
            v_proj(xv_ts, range(8, ST))

            def alloc_psA(qc):
                return [pb.tile([128, 512], dt.float32, tag="pb", name=f"att_{qc}_{h}")
                        for h in range(HC)]

            psA = alloc_psA(0)
            attention(0, range(0, 8), psA, pre_ex=pre_ex)
            for qc in range(1, QC):
                psA = alloc_psA(qc)
                attention(qc, range(0, 8), psA)
                tail(qc - 1)
            tail(QC - 1, last=True)

    nc.finalize()
    return nc


def _get_program():
    global _PROGRAM
    if _PROGRAM is None:
        _PROGRAM = _build_program()
    return _PROGRAM


def _prep_core_inputs(x_q, x_k, x_v, wq, bq, wk, bk, wv, bv, wo):
    """Build the 8 per-core input dicts (host-side shard + cast)."""
    ones_np = np.ones((128, 128), np.float32)
    xT = {}
    for b in range(2):
        xT[b] = (
            np.ascontiguousarray(x_q[b].T).astype(BF16),
            np.ascontiguousarray(x_k[b].T).astype(BF16),
            np.ascontiguousarray(x_v[b].T).astype(BF16),
        )
    in_maps = []
    for c in range(NCORES):
        b, g = c // 4, c % 4
        sl = slice(g * DC, (g + 1) * DC)
        wo_c = np.ascontiguousarray(
            wo[sl, :].reshape(HC, HD, D).transpose(1, 0, 2)
        ).astype(BF16)
        in_maps.append({
            "xqT": xT[b][0],
            "xkT": xT[b][1],
            "xvT": xT[b][2],
            "wq": wq[:, sl].astype(BF16),
            "wk": wk[:, sl].astype(BF16),
            "wv": wv[:, sl].astype(BF16),
            "wo": wo_c,
            "bq": np.ascontiguousarray(bq[sl].reshape(2, 128).T).astype(np.float32),
            "bk": np.ascontiguousarray(bk[sl].reshape(2, 128).T).astype(np.float32),
            "bv": np.broadcast_to(bv[sl], (128, DC)).astype(np.float32).copy(),
            "ones": ones_np,
        })
    return in_maps


def kernel(x_q, x_k, x_v, wq, bq, wk, bk, wv, bv, wo, bo):
    from concourse.bass_utils import run_bass_kernel_spmd

    x_q = np.asarray(x_q, np.float32)
    x_k = np.asarray(x_k, np.float32)
    x_v = np.asarray(x_v, np.float32)
    wq = np.asarray(wq, np.float32)
    wk = np.asarray(wk, np.float32)
    wv = np.asarray(wv, np.float32)
    wo = np.asarray(wo, np.float32)
    bq = np.asarray(bq, np.float32)
    bk = np.asarray(bk, np.float32)
    bv = np.asarray(bv, np.float32)
    bo = np.asarray(bo, np.float32)

    nc = _get_program()
    in_maps = _prep_core_inputs(x_q, x_k, x_v, wq, bq, wk, bk, wv, bv, wo)
    res = run_bass_kernel_spmd(nc, in_maps, list(range(NCORES)))

    out = np.zeros((2, S, D), np.float32)
    for c in range(NCORES):
        out[c // 4] += res.results[c]["out"]
    out += bo
    return out



# revision 3
# speedup vs baseline: 1.2737x; 1.2737x over previous
"""Trainium2 Bass kernel for nn_MultiHeadAttention (B=2, S=2048, D=1024, H=16).

Sharding: 8 cores = 2 batches x 4 head-groups. Core c handles batch c//4 and
heads [4*(c%4), 4*(c%4)+4). Each core computes its 4 heads' attention plus the
row-slice of the output projection; the host sums the 4 partial outputs per
batch and adds the output bias.

Dataflow (cost model: matmul = N_out cycles regardless of M/K, so every
matmul keeps M=128 / K=128 where the math allows):
  - qT/kT in [head_dim, seq] layout, 2 heads per 128-partition tile.
  - scoresT[kv, q] = kT.T @ qT per (head, kv-pair, q-chunk); exp on ScalarE
    (scale=1/8 folded in) -> ex tiles [128 kv, 1024] bf16. The exp stream is
    the ACT-side bottleneck and paces the steady-state windows.
  - attn[q, d+1] = ex.T @ [v | 1]: ex is the stationary operand (M=128 q,
    K=128 kv, N=65), accumulated over 16 kv tiles into PSUM [128, 4*65]
    per (head, q-chunk); col 64 of each head block = softmax denominator.
  - DVE reciprocal + per-partition tensor_scalar_mul normalizes into
    attn_n [128 q, 4*64] bf16 (q on partitions = denominators are
    per-partition scalars, no broadcast matmuls needed).
  - PE transpose (identity matmul) flips head-pairs [128 q, 128] ->
    [128 (2h*d), 128 q]; out = at.T @ wo accumulates K=128 (2 heads) per
    pass, halving the output projection.
  - Emission order software-pipelines windows: consume(qc) [attn@V+tail]
    interleaves with stage(qc+1) [scores+exp] per head so PE and ACT both
    stay dense; V/Q projections are spread into ACT-bound windows.

All matmuls run in bf16 (inputs cast on host) with fp32 PSUM accumulation.
"""

import sys

for _p in ("/opt/trn_rl_repo",):
    if _p not in sys.path:
        sys.path.insert(0, _p)

import numpy as np
import ml_dtypes

BF16 = ml_dtypes.bfloat16

S = 2048          # sequence length
D = 1024          # embed dim
HC = 4            # heads per core
HD = 64           # head dim
DC = HC * HD      # per-core projection width (256)
DT = D // 128     # D-tiles (8)
QC = S // 512     # q-chunks of 512 (4)
NKV = S // 128    # kv tiles of 128 (16)
NCORES = 8

_PROGRAM = None


def _build_program():
    import concourse.mybir as mybir
    import concourse.tile as tile
    from concourse import bacc

    dt = mybir.dt
    AF = mybir.ActivationFunctionType
    ALU = mybir.AluOpType

    nc = bacc.Bacc()

    xqT = nc.declare_dram_parameter("xqT", [D, S], dt.bfloat16, isOutput=False)
    xkT = nc.declare_dram_parameter("xkT", [D, S], dt.bfloat16, isOutput=False)
    xvT = nc.declare_dram_parameter("xvT", [D, S], dt.bfloat16, isOutput=False)
    wq = nc.declare_dram_parameter("wq", [D, DC], dt.bfloat16, isOutput=False)
    wk = nc.declare_dram_parameter("wk", [D, DC], dt.bfloat16, isOutput=False)
    wv = nc.declare_dram_parameter("wv", [D, DC], dt.bfloat16, isOutput=False)
    wo = nc.declare_dram_parameter("wo", [128, 2, D], dt.bfloat16, isOutput=False)
    bq = nc.declare_dram_parameter("bq", [128, 2], dt.float32, isOutput=False)
    bk = nc.declare_dram_parameter("bk", [128, 2], dt.float32, isOutput=False)
    bv = nc.declare_dram_parameter("bv", [128, DC], dt.float32, isOutput=False)
    ident = nc.declare_dram_parameter("ident", [128, 128], dt.bfloat16, isOutput=False)
    out = nc.declare_dram_parameter("out", [S, D], dt.float32, isOutput=True)

    out_t = out.rearrange("(t p) d -> t p d", p=128)

    with tile.TileContext(nc) as tc:
        with (
            tc.tile_pool(name="const", bufs=1) as cp,
            tc.tile_pool(name="xt", bufs=7) as xp,
            tc.tile_pool(name="expp", bufs=32) as ep,
            tc.tile_pool(name="anp", bufs=8) as np_,
            tc.tile_pool(name="atp", bufs=4) as ap_,
            tc.tile_pool(name="rcp", bufs=2) as rp,
            tc.tile_pool(name="outp", bufs=3) as op_,
            tc.tile_pool(name="pa", bufs=2, space="PSUM") as pa,
            tc.tile_pool(name="pacc", bufs=2, space="PSUM") as pacc,
            tc.tile_pool(name="pmix", bufs=2, space="PSUM") as pm,
        ):
            # ---- persistent tiles ----
            wq_sb = cp.tile([128, DT, DC], dt.bfloat16, tag="wq_sb")
            wk_sb = cp.tile([128, DT, DC], dt.bfloat16, tag="wk_sb")
            wv_sb = cp.tile([128, DT, DC], dt.bfloat16, tag="wv_sb")
            wo_sb = cp.tile([128, 2, D], dt.bfloat16, tag="wo_sb")
            bq_sb = cp.tile([128, 2], dt.float32, tag="bq_sb")
            bk_sb = cp.tile([128, 2], dt.float32, tag="bk_sb")
            bv_sb = cp.tile([128, DC], dt.float32, tag="bv_sb")
            id_sb = cp.tile([128, 128], dt.bfloat16, tag="id_sb")

            qT_sb = [cp.tile([128, 2, 512], dt.bfloat16, tag=f"qT{i}", name=f"qT{i}")
                     for i in range(QC)]
            kT_sb = [cp.tile([128, 2, 512], dt.bfloat16, tag=f"kT{i}", name=f"kT{i}")
                     for i in range(QC)]
            # v' blocks of 65 per head: v cols 0..63, ones col 64
            v_sb = [cp.tile([128, HC * 65], dt.bfloat16, tag=f"v{i}", name=f"v{i}")
                    for i in range(NKV)]

            xq_t: list = [None] * 4
            xk_t: list = [None] * 4
            xv_t: list = [None] * 4

            def dma_x(xT, arr, q, nm):
                t = xp.tile([128, DT, 512], dt.bfloat16, tag="xt", name=f"x_{nm}{q}")
                nc.sync.dma_start(
                    t[:],
                    xT.rearrange("(t p) s -> p t s", p=128)[:, :, q * 512:(q + 1) * 512])
                arr[q] = t

            def kq_proj(xts, w_sb, dst, b_sb, qc, pts=(0, 1)):
                for pt in pts:
                    ps = pm.tile([128, 512], dt.float32, tag="pm", name=f"pp{qc}_{pt}")
                    for Dti in range(DT):
                        nc.tensor.matmul(
                            ps[:],
                            w_sb[:, Dti, pt * 128:(pt + 1) * 128],
                            xts[qc][:, Dti, :],
                            start=(Dti == 0),
                            stop=(Dti == DT - 1),
                        )
                    nc.vector.tensor_scalar_add(dst[qc][:, pt, :], ps[:], b_sb[:, pt:pt + 1])

            def v_chain(st, h):
                q, off = st // 4, (st % 4) * 128
                ps = pm.tile([128, HD], dt.float32, tag="pm", name=f"vp{st}_{h}")
                for Dti in range(DT):
                    nc.tensor.matmul(
                        ps[:],
                        xv_t[q][:, Dti, off:off + 128],
                        wv_sb[:, Dti, h * HD:(h + 1) * HD],
                        start=(Dti == 0),
                        stop=(Dti == DT - 1),
                    )
                nc.vector.tensor_tensor(
                    v_sb[st].rearrange("p (h c) -> p h c", c=65)[:, h, 0:64],
                    ps[:], bv_sb[:, h * HD:(h + 1) * HD], ALU.add)

            exs = {}

            def se(qc, h, kvb):
                pt, lo = h // 2, (h % 2) * 64
                scp = pa.tile([128, 1024], dt.float32, tag="pa", name=f"sc{qc}_{h}_{kvb}")
                for j in range(2):
                    kt = kvb * 2 + j
                    nc.tensor.matmul(
                        scp[:, j * 512:(j + 1) * 512],
                        kT_sb[kt // 4][lo:lo + 64, pt, (kt % 4) * 128:(kt % 4 + 1) * 128],
                        qT_sb[qc][lo:lo + 64, pt, :],
                        start=True,
                        stop=True,
                    )
                ex = ep.tile([128, 1024], dt.bfloat16, tag="ex", name=f"ex{qc}_{h}_{kvb}")
                nc.scalar.activation(ex[:], scp[:], AF.Exp, scale=0.125)
                exs[(qc, h, kvb)] = ex

            attn_n = {}

            def burst(qc, h):
                # attn[q, d]+sums for (qc, h), all 4 q-subtiles, K accumulated
                # over the 16 kv tiles; ex is the stationary operand.
                acc = pacc.tile([128, HC * 65], dt.float32, tag="acc", name=f"acc{qc}_{h}")
                for j in range(4):
                    for kt in range(NKV):
                        e = exs[(qc, h, kt // 2)]
                        o = (kt % 2) * 512 + j * 128
                        nc.tensor.matmul(
                            acc[:, j * 65:j * 65 + 65],
                            e[:, o:o + 128],
                            v_sb[kt][:, h * 65:(h + 1) * 65],
                            start=(kt == 0),
                            stop=(kt == NKV - 1),
                        )
                for kvb in range(8):
                    exs.pop((qc, h, kvb))
                if h == 0:
                    for j in range(4):
                        attn_n[(qc, j)] = np_.tile(
                            [128, 256], dt.bfloat16, tag="an", name=f"an{qc}_{j}")
                accv = acc.rearrange("p (j c) -> p j c", c=65)
                rc = rp.tile([128, HC], dt.float32, tag="rc", name=f"rc{qc}_{h}")
                nc.vector.reciprocal(rc[:], accv[:, :, 64])
                for j in range(4):
                    nc.vector.tensor_scalar_mul(
                        attn_n[(qc, j)][:, h * 64:(h + 1) * 64],
                        accv[:, j, 0:64], rc[:, j:j + 1])

            def tail_j(qc, j):
                # transpose head pairs of q-subtile j, then the output
                # projection row-slice for s-tile qc*4+j
                att = attn_n[(qc, j)]
                att_t = ap_.tile([128, 256], dt.bfloat16, tag="at", name=f"at{qc}_{j}")
                for hp in range(2):
                    tr = pm.tile([128, 128], dt.bfloat16, tag="pm", name=f"tr{qc}_{j}_{hp}")
                    nc.tensor.transpose(tr[:], att[:, hp * 128:(hp + 1) * 128], id_sb[:])
                    nc.vector.tensor_copy(att_t[:, hp * 128:(hp + 1) * 128], tr[:])
                st = qc * 4 + j
                o_sb = op_.tile([128, D], dt.float32, tag="osb", name=f"o{st}")
                for dc2 in range(2):
                    po = pm.tile([128, 512], dt.float32, tag="pm", name=f"po{st}_{dc2}")
                    for hp in range(2):
                        nc.tensor.matmul(
                            po[:],
                            att_t[:, hp * 128:(hp + 1) * 128],
                            wo_sb[:, hp, dc2 * 512:(dc2 + 1) * 512],
                            start=(hp == 0),
                            stop=(hp == 1),
                        )
                    nc.vector.tensor_copy(o_sb[:, dc2 * 512:(dc2 + 1) * 512], po[:])
                    nc.sync.dma_start(
                        out_t[st][:, dc2 * 512:(dc2 + 1) * 512],
                        o_sb[:, dc2 * 512:(dc2 + 1) * 512])

            # ---- DMA emission (SP queue, consumption order) ----
            nc.sync.dma_start(wk_sb[:], wk.rearrange("(t p) m -> p t m", p=128))
            nc.sync.dma_start(bk_sb[:], bk[:])
            dma_x(xkT, xk_t, 0, "k")
            nc.sync.dma_start(wq_sb[:], wq.rearrange("(t p) m -> p t m", p=128))
            nc.sync.dma_start(bq_sb[:], bq[:])
            dma_x(xqT, xq_t, 0, "q")
            dma_x(xkT, xk_t, 1, "k")
            dma_x(xkT, xk_t, 2, "k")
            dma_x(xkT, xk_t, 3, "k")
            dma_x(xqT, xq_t, 1, "q")
            nc.sync.dma_start(wv_sb[:], wv.rearrange("(t p) m -> p t m", p=128))
            nc.sync.dma_start(bv_sb[:], bv[:])
            nc.sync.dma_start(wo_sb[:], wo[:])
            nc.sync.dma_start(id_sb[:], ident[:])
            for q in range(4):
                dma_x(xvT, xv_t, q, "v")
            dma_x(xqT, xq_t, 2, "q")
            dma_x(xqT, xq_t, 3, "q")

            for st in range(NKV):
                nc.vector.memset(
                    v_sb[st].rearrange("p (h c) -> p h c", c=65)[:, :, 64:65], 1.0)

            # ---- window 0: projections + stage(0) + vchains h0 ----
            kq_proj(xk_t, wk_sb, kT_sb, bk_sb, 0)
            kq_proj(xq_t, wq_sb, qT_sb, bq_sb, 0)
            se(0, 0, 0); se(0, 0, 1)
            kq_proj(xk_t, wk_sb, kT_sb, bk_sb, 1)
            se(0, 0, 2); se(0, 0, 3)
            kq_proj(xk_t, wk_sb, kT_sb, bk_sb, 2)
            se(0, 0, 4); se(0, 0, 5)
            kq_proj(xk_t, wk_sb, kT_sb, bk_sb, 3)
            se(0, 0, 6); se(0, 0, 7)
            kq_proj(xq_t, wq_sb, qT_sb, bq_sb, 1)
            vst = 0
            for h in range(1, HC):
                for kvb in range(8):
                    se(0, h, kvb)
                    if vst < NKV:
                        v_chain(vst, 0)
                        vst += 1
                    if vst < NKV:
                        v_chain(vst, 0)
                        vst += 1

            # ---- steady windows: consume(qc) + stage(qc+1) ----
            for qc in range(QC):
                nq = qc + 1
                for h in range(HC):
                    if qc == 0 and h >= 1:
                        for st in range(NKV):
                            v_chain(st, h)
                    burst(qc, h)
                    if h < 3:
                        if nq < QC:
                            for kvb in range(8):
                                se(nq, h, kvb)
                        if qc < 2 and h == 1:
                            # qT for stage(qc+2), pt0 feeds its h0/h1 blocks
                            kq_proj(xq_t, wq_sb, qT_sb, bq_sb, qc + 2, pts=(0,))
                    else:
                        # h == 3: interleave next-window scores with the tail
                        ses = [(nq, 3, kvb) for kvb in range(8)] if nq < QC else []
                        si = 0
                        for j in range(4):
                            for _ in range(2):
                                if si < len(ses):
                                    se(*ses[si]); si += 1
                            tail_j(qc, j)
                        while si < len(ses):
                            se(*ses[si]); si += 1
                        if qc < 2:
                            kq_proj(xq_t, wq_sb, qT_sb, bq_sb, qc + 2, pts=(1,))

    nc.finalize()
    return nc


def _get_program():
    global _PROGRAM
    if _PROGRAM is None:
        _PROGRAM = _build_program()
    return _PROGRAM


def _prep_core_inputs(x_q, x_k, x_v, wq, bq, wk, bk, wv, bv, wo):
    """Build the 8 per-core input dicts (host-side shard + cast)."""
    ident_np = np.eye(128, dtype=np.float32).astype(BF16)
    xT = {}
    for b in range(2):
        xT[b] = (
            np.ascontiguousarray(x_q[b].T).astype(BF16),
            np.ascontiguousarray(x_k[b].T).astype(BF16),
            np.ascontiguousarray(x_v[b].T).astype(BF16),
        )
    in_maps = []
    for c in range(NCORES):
        b, g = c // 4, c % 4
        sl = slice(g * DC, (g + 1) * DC)
        # wo rows for this head group, stacked per head pair: row hh*64+d of
        # pair hp = wo row for head 2*hp+hh, dim d
        wo_c = np.ascontiguousarray(
            wo[sl, :].reshape(2, 2, HD, D).transpose(1, 2, 0, 3).reshape(128, 2, D)
        ).astype(BF16)
        in_maps.append({
            "xqT": xT[b][0],
            "xkT": xT[b][1],
            "xvT": xT[b][2],
            "wq": wq[:, sl].astype(BF16),
            "wk": wk[:, sl].astype(BF16),
            "wv": wv[:, sl].astype(BF16),
            "wo": wo_c,
            "bq": np.ascontiguousarray(bq[sl].reshape(2, 128).T).astype(np.float32),
            "bk": np.ascontiguousarray(bk[sl].reshape(2, 128).T).astype(np.float32),
            "bv": np.broadcast_to(bv[sl], (128, DC)).astype(np.float32).copy(),
            "ident": ident_np,
        })
    return in_maps


def kernel(x_q, x_k, x_v, wq, bq, wk, bk, wv, bv, wo, bo):
    from concourse.bass_utils import run_bass_kernel_spmd

    x_q = np.asarray(x_q, np.float32)
    x_k = np.asarray(x_k, np.float32)
    x_v = np.asarray(x_v, np.float32)
    wq = np.asarray(wq, np.float32)
    wk = np.asarray(wk, np.float32)
    wv = np.asarray(wv, np.float32)
    wo = np.asarray(wo, np.float32)
    bq = np.asarray(bq, np.float32)
    bk = np.asarray(bk, np.float32)
    bv = np.asarray(bv, np.float32)
    bo = np.asarray(bo, np.float32)

    nc = _get_program()
    in_maps = _prep_core_inputs(x_q, x_k, x_v, wq, bq, wk, bk, wv, bv, wo)
    res = run_bass_kernel_spmd(nc, in_maps, list(range(NCORES)))

    out = np.zeros((2, S, D), np.float32)
    for c in range(NCORES):
        out[c // 4] += res.results[c]["out"]
    out += bo
    return out
